# revision 8
# baseline (speedup 1.0000x reference)
"""DeepseekV3 mini MoE MLP on 8 TRN2 NeuronCores.

Expert-parallel with load balancing. The router runs on host (0.1% of
FLOPs, mirrors the reference bit-for-bit); tokens are dispatched on the
host to per-core batches (the "all-to-all"), and the weighted combine
(scatter-add) also happens on the host.

Each core runs one fused gate/up/silu/mul/down program over C tokens in
bf16 (full TensorE rate, half the DMA/SBUF of f32r). A core's tokens
come from up to TWO experts: a "main" slot of capA tokens (one expert's
queue) and a small "spill" slot of capB tokens holding overflow from
any over-loaded expert. Both experts' weights are SBUF-resident for the
whole kernel (bf16 makes two full sets fit). This packs the 65536
token-expert pairs into C = capA+capB ~= 8320 per core instead of
padding every core to the max expert load (8704) -- a ~4.5% cut in PE
time, which dominates.

Layouts are feature-major ([dim, tokens]) so every matmul contracts
over the SBUF partition dim with no transposes anywhere on device.
Weights are passed pre-chunked ([HT, P, DT, P]) so each output-column
block's weights arrive in one fully-contiguous DMA; x tiles arrive as 8
independent per-chunk DMAs so the first matmul starts ~1us in.
"""

import numpy as np
import ml_dtypes

import concourse.bass as bass
import concourse.mybir as mybir
import concourse.tile as tile
from concourse import bacc
from concourse.bass_utils import run_bass_kernel_spmd

DIM = 1024
HIDDEN = 1024
NUM_EXPERTS = 8
TOP_K = 2
P = 128
TT = 512  # main token tile (PSUM bank = 512 fp32)
DT = DIM // P  # 8 d-tiles
HT = HIDDEN // P  # 8 h-tiles

F32 = mybir.dt.float32
BF16 = mybir.dt.bfloat16
NPBF16 = ml_dtypes.bfloat16

_program_cache: dict[tuple, object] = {}
LAST_RESULT = None


def _tiles_of(cap: int) -> tuple:
    full, rem = divmod(cap, TT)
    return (TT,) * full + ((rem,) if rem else ())


def _build_program(tilesA: tuple, tilesB: tuple):
    """Fused MoE FFN over C = sum(tilesA)+sum(tilesB) tokens.

    Tiles in tilesA use weight set A, tiles in tilesB use set B.

    DRAM params (per core):
      xt [DIM, C]              tokens, transposed (d-major); A tokens
                               first, then B tokens
      wg*/wu* [HT, P, DT, P]   gate/up proj, chunked [h-blk, d-in, d-blk, h-in]
      wd* [DT, P, HT, P]       down proj, chunked [dout-blk, h-in, h-blk, dout-in]
      yt [DIM, C]              output, transposed, f32
    """
    C = sum(tilesA) + sum(tilesB)
    nc = bacc.Bacc(None, target_bir_lowering=False, debug=False)
    xt = nc.declare_dram_parameter("xt", [DIM, C], BF16, isOutput=False)
    w_dram = {}
    for s in "AB":
        w_dram["wg" + s] = nc.declare_dram_parameter(
            "wg" + s, [HT, P, DT, P], BF16, isOutput=False
        )
        w_dram["wu" + s] = nc.declare_dram_parameter(
            "wu" + s, [HT, P, DT, P], BF16, isOutput=False
        )
        w_dram["wd" + s] = nc.declare_dram_parameter(
            "wd" + s, [DT, P, HT, P], BF16, isOutput=False
        )
    yt = nc.declare_dram_parameter("yt", [DIM, C], BF16, isOutput=True)

    tiles = [(t, 0) for t in tilesA] + [(t, 1) for t in tilesB]

    with tile.TileContext(nc) as tc:
        with (
            tc.tile_pool(name="wpool", bufs=1) as wpool,
            tc.tile_pool(name="xpool", bufs=2) as xpool,
            tc.tile_pool(name="hpool", bufs=2) as hpool,
            tc.tile_pool(name="apool", bufs=3) as apool,
            tc.tile_pool(name="ypool", bufs=2) as ypool,
            tc.tile_pool(name="pg", bufs=2, space="PSUM") as pgpool,
            tc.tile_pool(name="pu", bufs=2, space="PSUM") as pupool,
            tc.tile_pool(name="py", bufs=2, space="PSUM") as pypool,
        ):
            # SBUF-resident weight chunk tiles, one per output-column block.
            wg_c, wu_c, wd_c = [[], []], [[], []], [[], []]
            for si, s in enumerate("AB"):
                for k in range(HT):
                    wg_c[si].append(
                        wpool.tile([P, DT * P], BF16, name=f"wg{s}{k}", tag=f"wg{s}{k}")
                    )
                    wu_c[si].append(
                        wpool.tile([P, DT * P], BF16, name=f"wu{s}{k}", tag=f"wu{s}{k}")
                    )
                    wd_c[si].append(
                        wpool.tile([P, HT * P], BF16, name=f"wd{s}{k}", tag=f"wd{s}{k}")
                    )

            # First gate/up chunk DMAs issue on three queues in parallel:
            # the first matmul only needs wgA[0] + x chunk 0, so nothing
            # serializes behind a single issue queue.
            nc.sync.dma_start(out=wg_c[0][0][:, :], in_=w_dram["wgA"].ap()[0])
            nc.scalar.dma_start(out=wu_c[0][0][:, :], in_=w_dram["wuA"].ap()[0])

            off = 0
            for ti, (tt, si) in enumerate(tiles):
                ts = bass.ds(off, tt)
                off += tt
                s = "AB"[si]
                # x arrives as 8 independent per-d-chunk DMAs split across
                # the idle GpSimd queue and Sync: block-a matmuls only wait
                # on chunk a.
                x_c = []
                for a in range(DT):
                    xc = xpool.tile([P, TT], BF16, tag=f"x{a}")
                    q = nc.gpsimd if a < DT // 2 else nc.sync
                    q.dma_start(out=xc[:, :tt], in_=xt.ap()[a * P : (a + 1) * P, ts])
                    x_c.append(xc)
                if ti == 0:
                    # Remaining A-set chunks right behind tile 0's x: block-k
                    # matmuls start as soon as chunk k lands.
                    for k in range(1, HT):
                        nc.sync.dma_start(out=wg_c[0][k][:, :], in_=w_dram["wgA"].ap()[k])
                        nc.sync.dma_start(out=wu_c[0][k][:, :], in_=w_dram["wuA"].ap()[k])
                    for k in range(HT):
                        nc.sync.dma_start(out=wd_c[0][k][:, :], in_=w_dram["wdA"].ap()[k])
                elif ti == 1:
                    # B set is only needed for the spill tile at the very end.
                    for k in range(HT):
                        nc.gpsimd.dma_start(out=wg_c[1][k][:, :], in_=w_dram["wgB"].ap()[k])
                        nc.gpsimd.dma_start(out=wu_c[1][k][:, :], in_=w_dram["wuB"].ap()[k])
                        nc.gpsimd.dma_start(out=wd_c[1][k][:, :], in_=w_dram["wdB"].ap()[k])

                h_sb = hpool.tile([P, HT * TT], BF16, tag="h")
                for h in range(HT):
                    pg = pgpool.tile([P, tt], F32, tag="pg")
                    pu = pupool.tile([P, tt], F32, tag="pu")
                    for a in range(DT):
                        nc.tensor.matmul(
                            pg[:, :],
                            wg_c[si][h][:, a * P : (a + 1) * P],
                            x_c[a][:, :tt],
                            start=(a == 0),
                            stop=(a == DT - 1),
                        )
                    for a in range(DT):
                        nc.tensor.matmul(
                            pu[:, :],
                            wu_c[si][h][:, a * P : (a + 1) * P],
                            x_c[a][:, :tt],
                            start=(a == 0),
                            stop=(a == DT - 1),
                        )
                    sil_sb = apool.tile([P, TT], F32, tag="sil")
                    nc.scalar.activation(
                        sil_sb[:, :tt], pg[:, :], mybir.ActivationFunctionType.Silu
                    )
                    nc.vector.tensor_tensor(
                        h_sb[:, h * TT : h * TT + tt],
                        sil_sb[:, :tt],
                        pu[:, :],
                        mybir.AluOpType.mult,
                    )

                y_sb = ypool.tile([P, HT * TT], BF16, tag="y")
                for do in range(HT):
                    py = pypool.tile([P, tt], F32, tag="py")
                    for a in range(HT):
                        nc.tensor.matmul(
                            py[:, :],
                            wd_c[si][do][:, a * P : (a + 1) * P],
                            h_sb[:, a * TT : a * TT + tt],
                            start=(a == 0),
                            stop=(a == HT - 1),
                        )
                    nc.scalar.copy(y_sb[:, do * TT : do * TT + tt], py[:, :])
                nc.sync.dma_start(
                    out=yt.ap()[:, ts].rearrange("(b p) t -> p b t", p=P),
                    in_=y_sb[:, :].rearrange("p (b t) -> p b t", t=TT)[:, :, :tt],
                )
    nc.compile()
    return nc


def _get_program(tilesA: tuple, tilesB: tuple):
    key = (tilesA, tilesB)
    if key not in _program_cache:
        _program_cache[key] = _build_program(tilesA, tilesB)
    return _program_cache[key]


def _pack(counts: np.ndarray):
    """Choose per-core capacities (capA main slot, capB spill slot) and the
    spill-piece assignment. All 8 cores run one SPMD program of capacity
    C = capA + capB; expert e's queue fills core e's main slot, overflow is
    chopped into <=capB pieces placed in other cores' spill slots."""
    total = int(counts.sum())
    lb = ((total + 8 * 64 - 1) // (8 * 64)) * 64
    best = None
    for C in range(lb, lb + 4096, 64):
        for capB in range(64, min(TT, C - 64) + 1, 64):
            capA = C - capB
            pieces = sum(
                -(-max(0, int(n) - capA) // capB) for n in counts
            )
            if pieces <= 8:
                ntiles = -(-capA // TT) + 1
                cand = (C, ntiles, capB)
                if best is None or cand < best:
                    best = cand
        if best is not None and best[0] == C:
            break
    assert best is not None
    C, _, capB = best
    capA = C - capB
    # Spill pieces: (expert, start offset within expert queue, length)
    pieces = []
    for e in range(NUM_EXPERTS):
        r = int(counts[e]) - capA
        start = capA
        while r > 0:
            ln = min(r, capB)
            pieces.append((e, start, ln))
            start += ln
            r -= ln
    assert len(pieces) <= 8
    return capA, capB, pieces


def _chunk_w(wt: np.ndarray) -> np.ndarray:
    """[K, M] weight (K contracted) -> chunk layout [m_blk, k_in, k_blk, m_in],
    contiguous per m_blk."""
    K, M = wt.shape
    return np.ascontiguousarray(
        wt.reshape(K // P, P, M // P, P).transpose(2, 1, 0, 3).astype(NPBF16)
    )


def _route(flat: np.ndarray, gate_w: np.ndarray):
    """Mirror the reference router bit-for-bit (jax ops, same backend)."""
    try:
        import jax
        import jax.numpy as jnp

        logits = jnp.asarray(flat) @ jnp.asarray(gate_w).T
        scores = jax.nn.sigmoid(logits)
        top_val, top_idx = jax.lax.top_k(scores, TOP_K)
        top_val = top_val / (top_val.sum(-1, keepdims=True) + 1e-9)
        return np.asarray(top_val), np.asarray(top_idx)
    except Exception:
        # numpy fallback: identical selection semantics (stable descending)
        logits = flat @ gate_w.T
        scores = 1.0 / (1.0 + np.exp(-logits))
        order = np.argsort(-scores, axis=-1, kind="stable")
        top_idx = order[:, :TOP_K].astype(np.int32)
        top_val = np.take_along_axis(scores, top_idx, axis=-1)
        top_val = top_val / (top_val.sum(-1, keepdims=True) + 1e-9)
        return top_val.astype(np.float32), top_idx


def kernel(x, gate_w, gate_proj, up_proj, down_proj):
    x = np.asarray(x)
    bsz, seqlen, dim = x.shape
    flat = np.ascontiguousarray(x.reshape(-1, dim), dtype=np.float32)
    T = flat.shape[0]
    gate_w = np.asarray(gate_w, dtype=np.float32)
    gate_proj = np.asarray(gate_proj, dtype=np.float32)
    up_proj = np.asarray(up_proj, dtype=np.float32)
    down_proj = np.asarray(down_proj, dtype=np.float32)

    top_val, top_idx = _route(flat, gate_w)

    idx_list = []
    cw_list = []
    for e in range(NUM_EXPERTS):
        mask = top_idx == e  # [T, K]
        tok = np.nonzero(mask.any(axis=1))[0]
        w = (top_val * mask).sum(axis=1)[tok].astype(np.float32)
        idx_list.append(tok)
        cw_list.append(w)

    counts = np.array([len(i) for i in idx_list])
    capA, capB, pieces = _pack(counts)
    C = capA + capB
    tilesA, tilesB = _tiles_of(capA), _tiles_of(capB)
    nc = _get_program(tilesA, tilesB)

    flat_bf = flat.astype(NPBF16)
    wchunks = []
    for e in range(NUM_EXPERTS):
        wchunks.append(
            (
                _chunk_w(gate_proj[e].T),
                _chunk_w(up_proj[e].T),
                _chunk_w(down_proj[e].T),
            )
        )
    zero_w = (
        np.zeros((HT, P, DT, P), NPBF16),
        np.zeros((HT, P, DT, P), NPBF16),
        np.zeros((DT, P, HT, P), NPBF16),
    )

    in_maps = []
    spills = []  # per core: (expert, start, len) or None
    for c in range(NUM_EXPERTS):
        mA = min(len(idx_list[c]), capA)
        xt = np.zeros((DIM, C), dtype=NPBF16)
        xt[:, :mA] = flat_bf[idx_list[c][:mA]].T
        sp = pieces[c] if c < len(pieces) else None
        spills.append(sp)
        wB = zero_w
        if sp is not None:
            e, start, ln = sp
            xt[:, capA : capA + ln] = flat_bf[idx_list[e][start : start + ln]].T
            wB = wchunks[e]
        wA = wchunks[c]
        in_maps.append(
            {
                "xt": xt,
                "wgA": wA[0], "wuA": wA[1], "wdA": wA[2],
                "wgB": wB[0], "wuB": wB[1], "wdB": wB[2],
            }
        )

    res = run_bass_kernel_spmd(nc, in_maps, core_ids=list(range(NUM_EXPERTS)))
    global LAST_RESULT
    LAST_RESULT = res

    out = np.zeros((T, DIM), dtype=np.float32)
    for c in range(NUM_EXPERTS):
        yt = np.asarray(res.results[c]["yt"], dtype=np.float32)
        mA = min(len(idx_list[c]), capA)
        if mA:
            out[idx_list[c][:mA]] += (yt[:, :mA] * cw_list[c][None, :mA]).T
        sp = spills[c]
        if sp is not None:
            e, start, ln = sp
            tok = idx_list[e][start : start + ln]
            out[tok] += (yt[:, capA : capA + ln] * cw_list[e][None, start : start + ln]).T
    return out.reshape(bsz, seqlen, dim)


# revision 11
# speedup vs baseline: 1.0064x; 1.0064x over previous
"""DeepseekV3 mini MoE MLP on 8 TRN2 NeuronCores.

Expert-parallel with load balancing. The router runs on host (0.1% of
FLOPs, mirrors the reference bit-for-bit); tokens are dispatched on the
host to per-core batches (the "all-to-all"), and the weighted combine
(scatter-add) also happens on the host.

Each core runs one fused gate/up/silu/mul/down program over C tokens in
bf16 (full TensorE rate, half the DMA/SBUF of f32r). A core's tokens
come from up to TWO experts: a "main" slot of capA tokens (one expert's
queue) and a small "spill" slot of capB tokens holding overflow from
any over-loaded expert. Both experts' weights are SBUF-resident for the
whole kernel (bf16 makes two full sets fit). This packs the 65536
token-expert pairs into C = capA+capB ~= 8320 per core instead of
padding every core to the max expert load (8704) -- a ~4.5% cut in PE
time, which dominates.

Layouts are feature-major ([dim, tokens]) so every matmul contracts
over the SBUF partition dim with no transposes anywhere on device.
Weights are passed pre-chunked ([HT, P, DT, P]) so each output-column
block's weights arrive in one fully-contiguous DMA; x tiles arrive as 8
independent per-chunk DMAs so the first matmul starts ~1us in.
"""

import numpy as np
import ml_dtypes

import concourse.bass as bass
import concourse.mybir as mybir
import concourse.tile as tile
from concourse import bacc
from concourse.bass_utils import run_bass_kernel_spmd

DIM = 1024
HIDDEN = 1024
NUM_EXPERTS = 8
TOP_K = 2
P = 128
TT = 512  # main token tile (PSUM bank = 512 fp32)
DT = DIM // P  # 8 d-tiles
HT = HIDDEN // P  # 8 h-tiles

F32 = mybir.dt.float32
BF16 = mybir.dt.bfloat16
NPBF16 = ml_dtypes.bfloat16

_program_cache: dict[tuple, object] = {}
LAST_RESULT = None


def _tiles_of(cap: int) -> tuple:
    full, rem = divmod(cap, TT)
    return (TT,) * full + ((rem,) if rem else ())


def _build_program(tilesA: tuple, tilesB: tuple):
    """Fused MoE FFN over C = sum(tilesA)+sum(tilesB) tokens.

    Tiles in tilesA use weight set A, tiles in tilesB use set B.

    DRAM params (per core):
      xt [DIM, C]              tokens, transposed (d-major); A tokens
                               first, then B tokens
      wg*/wu* [HT, P, DT, P]   gate/up proj, chunked [h-blk, d-in, d-blk, h-in]
      wd* [DT, P, HT, P]       down proj, chunked [dout-blk, h-in, h-blk, dout-in]
      yt [DIM, C]              output, transposed, f32
    """
    C = sum(tilesA) + sum(tilesB)
    nc = bacc.Bacc(None, target_bir_lowering=False, debug=False)
    xt = nc.declare_dram_parameter("xt", [DIM, C], BF16, isOutput=False)
    w_dram = {}
    for s in "AB":
        w_dram["wg" + s] = nc.declare_dram_parameter(
            "wg" + s, [HT, P, DT, P], BF16, isOutput=False
        )
        w_dram["wu" + s] = nc.declare_dram_parameter(
            "wu" + s, [HT, P, DT, P], BF16, isOutput=False
        )
        w_dram["wd" + s] = nc.declare_dram_parameter(
            "wd" + s, [DT, P, HT, P], BF16, isOutput=False
        )
    yt = nc.declare_dram_parameter("yt", [DIM, C], BF16, isOutput=True)

    tiles = [(t, 0) for t in tilesA] + [(t, 1) for t in tilesB]

    with tile.TileContext(nc) as tc:
        with (
            tc.tile_pool(name="wpool", bufs=1) as wpool,
            tc.tile_pool(name="xpool", bufs=2) as xpool,
            tc.tile_pool(name="hpool", bufs=2) as hpool,
            tc.tile_pool(name="apool", bufs=3) as apool,
            tc.tile_pool(name="ypool", bufs=2) as ypool,
            tc.tile_pool(name="pg", bufs=2, space="PSUM") as pgpool,
            tc.tile_pool(name="pu", bufs=2, space="PSUM") as pupool,
            tc.tile_pool(name="py", bufs=2, space="PSUM") as pypool,
        ):
            # SBUF-resident weight chunk tiles, one per output-column block.
            wg_c, wu_c, wd_c = [[], []], [[], []], [[], []]
            for si, s in enumerate("AB"):
                for k in range(HT):
                    wg_c[si].append(
                        wpool.tile([P, DT * P], BF16, name=f"wg{s}{k}", tag=f"wg{s}{k}")
                    )
                    wu_c[si].append(
                        wpool.tile([P, DT * P], BF16, name=f"wu{s}{k}", tag=f"wu{s}{k}")
                    )
                    wd_c[si].append(
                        wpool.tile([P, HT * P], BF16, name=f"wd{s}{k}", tag=f"wd{s}{k}")
                    )

            # p-state pre-warm: ~3us of dummy matmuls on a zeroed tile so the
            # PE clock ramp (0.65/1.2 GHz -> 2.4 GHz over ~3us of activity)
            # burns while the first x/weight DMAs are still in flight.
            warm_sb = apool.tile([P, P], BF16, tag="warm")
            nc.vector.memset(warm_sb[:, :], 0)
            pw = pgpool.tile([P, P], F32, tag="warm")
            for _ in range(28):
                nc.tensor.matmul(pw[:, :], warm_sb[:, :], warm_sb[:, :],
                                 start=True, stop=True)

            # First gate/up chunk DMAs lead everything: the first matmul
            # only needs wgA[0] + x chunk 0.
            nc.sync.dma_start(out=wg_c[0][0][:, :], in_=w_dram["wgA"].ap()[0])
            nc.sync.dma_start(out=wu_c[0][0][:, :], in_=w_dram["wuA"].ap()[0])

            off = 0
            for ti, (tt, si) in enumerate(tiles):
                ts = bass.ds(off, tt)
                off += tt
                s = "AB"[si]
                # x arrives as 8 independent per-d-chunk DMAs: block-a
                # matmuls only wait on chunk a.
                x_c = []
                for a in range(DT):
                    xc = xpool.tile([P, TT], BF16, tag=f"x{a}")
                    nc.sync.dma_start(
                        out=xc[:, :tt], in_=xt.ap()[a * P : (a + 1) * P, ts]
                    )
                    x_c.append(xc)
                if ti == 0:
                    # Remaining A-set chunks right behind tile 0's x: block-k
                    # matmuls start as soon as chunk k lands.
                    for k in range(1, HT):
                        nc.sync.dma_start(out=wg_c[0][k][:, :], in_=w_dram["wgA"].ap()[k])
                        nc.sync.dma_start(out=wu_c[0][k][:, :], in_=w_dram["wuA"].ap()[k])
                    for k in range(HT):
                        nc.sync.dma_start(out=wd_c[0][k][:, :], in_=w_dram["wdA"].ap()[k])
                elif ti == 1:
                    # B set is only needed for the spill tile at the very end.
                    for k in range(HT):
                        nc.sync.dma_start(out=wg_c[1][k][:, :], in_=w_dram["wgB"].ap()[k])
                        nc.sync.dma_start(out=wu_c[1][k][:, :], in_=w_dram["wuB"].ap()[k])
                        nc.sync.dma_start(out=wd_c[1][k][:, :], in_=w_dram["wdB"].ap()[k])

                h_sb = hpool.tile([P, HT * TT], BF16, tag="h")
                for h in range(HT):
                    pg = pgpool.tile([P, tt], F32, tag="pg")
                    pu = pupool.tile([P, tt], F32, tag="pu")
                    for a in range(DT):
                        nc.tensor.matmul(
                            pg[:, :],
                            wg_c[si][h][:, a * P : (a + 1) * P],
                            x_c[a][:, :tt],
                            start=(a == 0),
                            stop=(a == DT - 1),
                        )
                    for a in range(DT):
                        nc.tensor.matmul(
                            pu[:, :],
                            wu_c[si][h][:, a * P : (a + 1) * P],
                            x_c[a][:, :tt],
                            start=(a == 0),
                            stop=(a == DT - 1),
                        )
                    sil_sb = apool.tile([P, TT], F32, tag="sil")
                    nc.scalar.activation(
                        sil_sb[:, :tt], pg[:, :], mybir.ActivationFunctionType.Silu
                    )
                    nc.vector.tensor_tensor(
                        h_sb[:, h * TT : h * TT + tt],
                        sil_sb[:, :tt],
                        pu[:, :],
                        mybir.AluOpType.mult,
                    )

                y_sb = ypool.tile([P, HT * TT], BF16, tag="y")
                for do in range(HT):
                    py = pypool.tile([P, tt], F32, tag="py")
                    for a in range(HT):
                        nc.tensor.matmul(
                            py[:, :],
                            wd_c[si][do][:, a * P : (a + 1) * P],
                            h_sb[:, a * TT : a * TT + tt],
                            start=(a == 0),
                            stop=(a == HT - 1),
                        )
                    nc.scalar.copy(y_sb[:, do * TT : do * TT + tt], py[:, :])
                nc.sync.dma_start(
                    out=yt.ap()[:, ts].rearrange("(b p) t -> p b t", p=P),
                    in_=y_sb[:, :].rearrange("p (b t) -> p b t", t=TT)[:, :, :tt],
                )
    nc.compile()
    return nc


def _get_program(tilesA: tuple, tilesB: tuple):
    key = (tilesA, tilesB)
    if key not in _program_cache:
        _program_cache[key] = _build_program(tilesA, tilesB)
    return _program_cache[key]


def _pack(counts: np.ndarray):
    """Choose per-core capacities (capA main slot, capB spill slot) and the
    spill-piece assignment. All 8 cores run one SPMD program of capacity
    C = capA + capB; expert e's queue fills core e's main slot, overflow is
    chopped into <=capB pieces placed in other cores' spill slots."""
    total = int(counts.sum())
    lb = ((total + 8 * 64 - 1) // (8 * 64)) * 64
    best = None
    for C in range(lb, lb + 4096, 64):
        for capB in range(64, min(TT, C - 64) + 1, 64):
            capA = C - capB
            pieces = sum(
                -(-max(0, int(n) - capA) // capB) for n in counts
            )
            if pieces <= 8:
                ntiles = -(-capA // TT) + 1
                cand = (C, ntiles, capB)
                if best is None or cand < best:
                    best = cand
        if best is not None and best[0] == C:
            break
    assert best is not None
    C, _, capB = best
    capA = C - capB
    # Spill pieces: (expert, start offset within expert queue, length)
    pieces = []
    for e in range(NUM_EXPERTS):
        r = int(counts[e]) - capA
        start = capA
        while r > 0:
            ln = min(r, capB)
            pieces.append((e, start, ln))
            start += ln
            r -= ln
    assert len(pieces) <= 8
    return capA, capB, pieces


def _chunk_w(wt: np.ndarray) -> np.ndarray:
    """[K, M] weight (K contracted) -> chunk layout [m_blk, k_in, k_blk, m_in],
    contiguous per m_blk."""
    K, M = wt.shape
    return np.ascontiguousarray(
        wt.reshape(K // P, P, M // P, P).transpose(2, 1, 0, 3).astype(NPBF16)
    )


def _route(flat: np.ndarray, gate_w: np.ndarray):
    """Mirror the reference router bit-for-bit (jax ops, same backend)."""
    try:
        import jax
        import jax.numpy as jnp

        logits = jnp.asarray(flat) @ jnp.asarray(gate_w).T
        scores = jax.nn.sigmoid(logits)
        top_val, top_idx = jax.lax.top_k(scores, TOP_K)
        top_val = top_val / (top_val.sum(-1, keepdims=True) + 1e-9)
        return np.asarray(top_val), np.asarray(top_idx)
    except Exception:
        # numpy fallback: identical selection semantics (stable descending)
        logits = flat @ gate_w.T
        scores = 1.0 / (1.0 + np.exp(-logits))
        order = np.argsort(-scores, axis=-1, kind="stable")
        top_idx = order[:, :TOP_K].astype(np.int32)
        top_val = np.take_along_axis(scores, top_idx, axis=-1)
        top_val = top_val / (top_val.sum(-1, keepdims=True) + 1e-9)
        return top_val.astype(np.float32), top_idx


def kernel(x, gate_w, gate_proj, up_proj, down_proj):
    x = np.asarray(x)
    bsz, seqlen, dim = x.shape
    flat = np.ascontiguousarray(x.reshape(-1, dim), dtype=np.float32)
    T = flat.shape[0]
    gate_w = np.asarray(gate_w, dtype=np.float32)
    gate_proj = np.asarray(gate_proj, dtype=np.float32)
    up_proj = np.asarray(up_proj, dtype=np.float32)
    down_proj = np.asarray(down_proj, dtype=np.float32)

    top_val, top_idx = _route(flat, gate_w)

    idx_list = []
    cw_list = []
    for e in range(NUM_EXPERTS):
        mask = top_idx == e  # [T, K]
        tok = np.nonzero(mask.any(axis=1))[0]
        w = (top_val * mask).sum(axis=1)[tok].astype(np.float32)
        idx_list.append(tok)
        cw_list.append(w)

    counts = np.array([len(i) for i in idx_list])
    capA, capB, pieces = _pack(counts)
    C = capA + capB
    tilesA, tilesB = _tiles_of(capA), _tiles_of(capB)
    nc = _get_program(tilesA, tilesB)

    flat_bf = flat.astype(NPBF16)
    wchunks = []
    for e in range(NUM_EXPERTS):
        wchunks.append(
            (
                _chunk_w(gate_proj[e].T),
                _chunk_w(up_proj[e].T),
                _chunk_w(down_proj[e].T),
            )
        )
    zero_w = (
        np.zeros((HT, P, DT, P), NPBF16),
        np.zeros((HT, P, DT, P), NPBF16),
        np.zeros((DT, P, HT, P), NPBF16),
    )

    in_maps = []
    spills = []  # per core: (expert, start, len) or None
    for c in range(NUM_EXPERTS):
        mA = min(len(idx_list[c]), capA)
        xt = np.zeros((DIM, C), dtype=NPBF16)
        xt[:, :mA] = flat_bf[idx_list[c][:mA]].T
        sp = pieces[c] if c < len(pieces) else None
        spills.append(sp)
        wB = zero_w
        if sp is not None:
            e, start, ln = sp
            xt[:, capA : capA + ln] = flat_bf[idx_list[e][start : start + ln]].T
            wB = wchunks[e]
        wA = wchunks[c]
        in_maps.append(
            {
                "xt": xt,
                "wgA": wA[0], "wuA": wA[1], "wdA": wA[2],
                "wgB": wB[0], "wuB": wB[1], "wdB": wB[2],
            }
        )

    res = run_bass_kernel_spmd(nc, in_maps, core_ids=list(range(NUM_EXPERTS)))
    global LAST_RESULT
    LAST_RESULT = res

    out = np.zeros((T, DIM), dtype=np.float32)
    for c in range(NUM_EXPERTS):
        yt = np.asarray(res.results[c]["yt"], dtype=np.float32)
        mA = min(len(idx_list[c]), capA)
        if mA:
            out[idx_list[c][:mA]] += (yt[:, :mA] * cw_list[c][None, :mA]).T
        sp = spills[c]
        if sp is not None:
            e, start, ln = sp
            tok = idx_list[e][start : start + ln]
            out[tok] += (yt[:, capA : capA + ln] * cw_list[e][None, start : start + ln]).T
    return out.reshape(bsz, seqlen, dim)
